# revision 2
# baseline (speedup 1.0000x reference)
"""Complex-magnitude MaxPool2d (k=2, s=2) Trainium2 Bass kernel.

Input  x:  [16, 2, 64, 224, 224] f32  (plane 0 = real, plane 1 = imag)
Output:    [16, 2, 64, 112, 112] f32  (value of the window element with the
                                       largest |z|^2 = re^2 + im^2)

Sharding: pure data parallel over batch: 16 / 8 cores = 2 examples per core.
Per core the 2(batch) x 64(channel) = 128 image planes map 1:1 onto the 128
SBUF partitions; DMA moves 28 image rows at a time in a single 128-partition
dma_start; compute runs on 14-row subchunks.

Selection reproduces jnp.argmax's first-index tie-break exactly:
horizontal pass first (left/even column wins ties via is_ge), then vertical
(top row wins ties).  norm2 = fl(fl(re*re)+fl(im*im)) in f32 — selection is
bit-exact with the reference.  The selected payload is rounded to f16 on
SBUF (rel err ~1e-4 << the 2e-2 gate) which halves output DMA traffic and
doubles/quadruples DVE throughput on the select arithmetic.

Engine split:
  ScalarE : squares (one ACT op per subchunk), horizontal select pre-fill,
            output dma issue (ACT HWDGE queue, parallel to input's SP queue)
  VectorE : norm add / is_ge / max as scalar_tensor_tensor (2x_2p DVE mode:
            2x faster than tensor_tensor at f32), horizontal
            copy_predicated, vertical select as f16 STT arithmetic
            (out = bot + m*(top-bot), 4x_2p DVE mode)
  DMA     : input chunks on the SP queue, f16 outputs on the ACT queue
"""

import numpy as np

import concourse.bass as bass
import concourse.mybir as mybir
from concourse import bacc, bass_utils, tile

# Per-core shard geometry (hardcoded; kernel.py must be self-contained).
NCORES = 8
B = 2            # batch per core
RI = 2           # real/imag planes
C = 64           # channels
H = W = 224
HO, WO = H // 2, W // 2
P = 128          # SBUF partitions = B * C
RD = 28          # image rows per DMA chunk
R = 14           # image rows per compute subchunk
SUB = RD // R    # compute subchunks per DMA chunk (2)
NCHUNK = H // RD  # 8
N = R * W        # free elements per plane per subchunk (3136)
GROUP = 4        # subchunks staged per output store (28 output rows)
SROWS = GROUP * (R // 2)

F32 = mybir.dt.float32
F16 = mybir.dt.float16
I8 = mybir.dt.uint8
OP = mybir.AluOpType
ACTF = mybir.ActivationFunctionType

_NC_CACHE = []


def _build_nc() -> bass.Bass:
    nc = bacc.Bacc("TRN2", target_bir_lowering=False, debug=False)
    # host pre-transposed: partition-major [b*c, ri, H, W] so every DMA is a
    # single-dim 128-partition transfer (hits all 16 SBUF AXI ports)
    x = nc.dram_tensor("x", [P, RI, H, W], F32, kind="ExternalInput").ap()
    out = nc.dram_tensor("out", [P, RI, HO, WO], F16, kind="ExternalOutput").ap()

    def stt(out, in0, in1, op):
        # (in0 * 1.0) `op` in1 on DVE — InstTensorScalarPtr supports the
        # 2x_2p/4x_2p high-perf modes that plain tensor_tensor lacks
        return nc.vector.scalar_tensor_tensor(
            out=out, in0=in0, scalar=1.0, in1=in1, op0=OP.mult, op1=op
        )

    with tile.TileContext(nc) as tc:
        with tc.tile_pool(name="pool", bufs=2) as pool:
            stage = None
            subidx = 0
            for k in range(NCHUNK):
                r0 = k * RD
                # xri free layout per partition: [ri][row 0..RD)[col]
                xri = pool.tile([P, RI * RD * W], F32, tag="xri")
                nc.sync.dma_start(
                    out=xri.rearrange("p (ri f) -> p ri f", ri=RI),
                    in_=x[:, :, r0 : r0 + RD, :].rearrange("p ri r w -> p ri (r w)"),
                )

                for s in range(SUB):
                    # subchunk views: rows rs..rs+R of each plane
                    xri6 = xri.rearrange(
                        "p (ri r w t) -> p ri r w t", ri=RI, r=RD, w=WO, t=2
                    )[:, :, s * R : (s + 1) * R, :, :]

                    # squares of re+im rows in one ACT op; norm2 via STT add
                    sqri = pool.tile([P, RI * N], F32, tag="sqri")
                    nc.scalar.activation(
                        out=sqri.rearrange(
                            "p (ri r w t) -> p ri r w t", ri=RI, r=R, w=WO, t=2
                        ),
                        in_=xri6,
                        func=ACTF.Square,
                    )
                    nrm = sqri[:, :N]
                    stt(nrm, nrm, sqri[:, N:], OP.add)

                    nrm4 = nrm.rearrange("p (r w t) -> p r w t", r=R, w=WO, t=2)
                    nE, nO = nrm4[:, :, :, 0], nrm4[:, :, :, 1]

                    # horizontal mask (contiguous u8): even/left wins ties
                    cH = pool.tile([P, R * WO], I8, tag="cH")
                    cH3 = cH.rearrange("p (r w) -> p r w", r=R, w=WO)
                    stt(cH3, nE, nO, OP.is_ge)
                    # horizontal norm max -> nrm odd slots (in place)
                    stt(nO, nE, nO, OP.max)

                    # horizontal select of (re, im) together into an f16
                    # tile: pre-fill with odd/right (ACT, casts f32->f16),
                    # overwrite with even/left where cH
                    riH = pool.tile([P, RI * R * WO], F16, tag="riH")
                    riH4 = riH.rearrange("p (ri r w) -> p ri r w", ri=RI, r=R, w=WO)
                    nc.scalar.copy(out=riH4, in_=xri6[:, :, :, :, 1])
                    cHb = cH3.unsqueeze(1).broadcast_to([P, RI, R, WO])
                    nc.vector.copy_predicated(
                        out=riH4, mask=cHb, data=xri6[:, :, :, :, 0]
                    )

                    # vertical mask from the horizontal maxes: top wins ties
                    nrm5 = nrm.rearrange(
                        "p (rp rt w t) -> p rp rt w t", rp=R // 2, rt=2, w=WO, t=2
                    )
                    cV = pool.tile([P, (R // 2) * WO], F16, tag="cV")
                    cV3 = cV.rearrange("p (rp w) -> p rp w", rp=R // 2, w=WO)
                    stt(cV3, nrm5[:, :, 0, :, 1], nrm5[:, :, 1, :, 1], OP.is_ge)

                    # vertical select into the staged output tile:
                    # dst = bot + cV*(top - bot), all-f16 STT (4x DVE mode)
                    riH5 = riH.rearrange(
                        "p (ri rp rt w) -> p ri rp rt w",
                        ri=RI, rp=R // 2, rt=2, w=WO,
                    )
                    top, bot = riH5[:, :, :, 0, :], riH5[:, :, :, 1, :]
                    if subidx % GROUP == 0:
                        stage = pool.tile([P, RI * SROWS * WO], F16, tag="stage")
                    stage4 = stage.rearrange(
                        "p (ri r w) -> p ri r w", ri=RI, r=SROWS, w=WO
                    )
                    s0 = (subidx % GROUP) * (R // 2)
                    dst = stage4[:, :, s0 : s0 + R // 2, :]

                    vt = pool.tile([P, RI * (R // 2) * WO], F16, tag="vt")
                    vt4 = vt.rearrange(
                        "p (ri rp w) -> p ri rp w", ri=RI, rp=R // 2, w=WO
                    )
                    cVb = cV3.unsqueeze(1).broadcast_to([P, RI, R // 2, WO])
                    stt(vt4, top, bot, OP.subtract)
                    stt(vt4, vt4, cVb, OP.mult)
                    stt(dst, vt4, bot, OP.add)

                    if (subidx + 1) % GROUP == 0:
                        g0 = (subidx + 1 - GROUP) * (R // 2)
                        # output rides the ACT HWDGE queue, leaving the SP
                        # queue free to stream input
                        nc.scalar.dma_start(
                            out=out[:, :, g0 : g0 + SROWS, :].rearrange(
                                "p ri r w -> p ri (r w)"
                            ),
                            in_=stage.rearrange("p (ri f) -> p ri f", ri=RI),
                        )
                    subidx += 1
    nc.compile()
    return nc


def get_nc() -> bass.Bass:
    if not _NC_CACHE:
        _NC_CACHE.append(_build_nc())
    return _NC_CACHE[0]


def kernel(x: np.ndarray, **run_kwargs) -> np.ndarray:
    nc = get_nc()
    xs = np.asarray(x, dtype=np.float32)
    assert xs.shape == (NCORES * B, RI, C, H, W), xs.shape
    # [16,2,64,H,W] -> per core [b,c,ri,H,W] flattened to [128,ri,H,W]
    xt = np.ascontiguousarray(xs.transpose(0, 2, 1, 3, 4))
    in_maps = [
        {"x": xt[B * i : B * (i + 1)].reshape(P, RI, H, W)} for i in range(NCORES)
    ]
    res = bass_utils.run_bass_kernel_spmd(
        nc, in_maps, core_ids=list(range(NCORES)), **run_kwargs
    )
    # per-core [128,ri,HO,WO] f16 -> [b,c,ri,HO,WO] -> [b,ri,c,HO,WO]
    out = np.concatenate(
        [
            np.asarray(res.results[i]["out"])
            .reshape(B, C, RI, HO, WO)
            .transpose(0, 2, 1, 3, 4)
            for i in range(NCORES)
        ],
        axis=0,
    )
    if run_kwargs:
        kernel.last_results = res
    return np.ascontiguousarray(out.astype(np.float32))


# revision 7
# speedup vs baseline: 1.2456x; 1.2456x over previous
"""Complex-magnitude MaxPool2d (k=2, s=2) Trainium2 Bass kernel.

Input  x:  [16, 2, 64, 224, 224] f32  (plane 0 = real, plane 1 = imag)
Output:    [16, 2, 64, 112, 112] f32  (value of the window element with the
                                       largest |z|^2 = re^2 + im^2)

Sharding: pure data parallel over batch: 16 / 8 cores = 2 examples per core.
Per core the 2(batch) x 64(channel) = 128 image planes map 1:1 onto the 128
SBUF partitions; compute runs on 14-row subchunks.

Selection reproduces jnp.argmax's first-index tie-break exactly:
horizontal pass first (left/even column wins ties via is_ge), then vertical
(top row wins ties via a strict bottom-wins is_gt on the in-place select).
norm2 = fl(fl(re*re)+fl(im*im)) in f32 - selection is bit-exact with the
reference.  The selected payload is rounded to f16 (rel err ~1e-4 << the
2e-2 gate), halving output DMA traffic.

Pipeline notes (v4):
 - No V-select pre-fill: CP-V writes bottom-candidates onto the top slots
   of riH in place where the bottom strictly wins; the output DMA then
   reads the winning rows straight out of riH (strided, 224B runs).
   This removes the ACT prefV op AND the ACT-behind-DVE dependency that
   serialized square(k+1) behind CP-H(k) in the previous version: the ACT
   stream (square, prefH) now depends only on DMA + buffer recycling, so
   it runs a subchunk ahead and DVE never starves.
 - First input DMA is a half chunk (14 rows) to cut pipeline fill time.
 - Output DMAs ride the ACT HWDGE queue, input the SP queue.
"""

import numpy as np

import concourse.bass as bass
import concourse.mybir as mybir
from concourse import bacc, bass_utils, tile

# Per-core shard geometry (hardcoded; kernel.py must be self-contained).
NCORES = 8
B = 2            # batch per core
RI = 2           # real/imag planes
C = 64           # channels
H = W = 224
HO, WO = H // 2, W // 2
P = 128          # SBUF partitions = B * C
R = 14           # image rows per compute subchunk
NSUB = H // R    # 16 subchunks
N = R * W        # free elements per plane per subchunk (3136)

F32 = mybir.dt.float32
F16 = mybir.dt.float16
I8 = mybir.dt.uint8
OP = mybir.AluOpType
ACTF = mybir.ActivationFunctionType

_NC_CACHE = []


def _build_nc() -> bass.Bass:
    nc = bacc.Bacc("TRN2", target_bir_lowering=False, debug=False)
    # host pre-transposed: partition-major [b*c, ri, H, W] so every DMA is a
    # single-dim 128-partition transfer (hits all 16 SBUF AXI ports)
    x = nc.dram_tensor("x", [P, RI, H, W], F32, kind="ExternalInput").ap()
    out = nc.dram_tensor("out", [P, RI, HO, WO], F16, kind="ExternalOutput").ap()

    with tile.TileContext(nc) as tc:
        with tc.tile_pool(name="pool", bufs=2) as pool:
            xtiles = {}

            def emit_chunk_dma(sc):
                t = pool.tile([P, RI * N], F32, tag="xri", bufs=3)
                nc.sync.dma_start(
                    out=t.rearrange("p (ri f) -> p ri f", ri=RI),
                    in_=x[:, :, sc * R : (sc + 1) * R, :].rearrange(
                        "p ri r w -> p ri (r w)"
                    ),
                )
                xtiles[sc] = t

            emit_chunk_dma(0)
            emit_chunk_dma(1)

            for sc in range(NSUB):
                t = xtiles[sc]
                # prefetch: issue the next chunk's DMA as early as possible
                if sc + 2 < NSUB:
                    emit_chunk_dma(sc + 2)

                # subchunk views: [ri, r, w, t]
                xri6 = t.rearrange(
                    "p (ri r w two) -> p ri r w two", ri=RI, r=R, w=WO, two=2
                )

                # squares of re+im rows in one ACT op; norm2 in place over
                # the re half
                sqri = pool.tile([P, RI * N], F32, tag="sqri")
                nc.scalar.activation(
                    out=sqri.rearrange(
                        "p (ri r w two) -> p ri r w two", ri=RI, r=R, w=WO, two=2
                    ),
                    in_=xri6,
                    func=ACTF.Square,
                )

                # horizontal select pre-fill with the odd/right candidate
                # (ACT, casts f32->f16); only depends on the DMA + buffers,
                # so the ACT stream never waits on DVE
                riH = pool.tile([P, RI * R * WO], F16, tag="riH")
                riH4 = riH.rearrange("p (ri r w) -> p ri r w", ri=RI, r=R, w=WO)
                nc.scalar.copy(out=riH4, in_=xri6[:, :, :, :, 1])

                nrm = sqri[:, :N]
                nc.vector.tensor_tensor(
                    out=nrm, in0=nrm, in1=sqri[:, N:], op=OP.add
                )

                nrm4 = nrm.rearrange("p (r w two) -> p r w two", r=R, w=WO, two=2)
                nE, nO = nrm4[:, :, :, 0], nrm4[:, :, :, 1]

                # horizontal mask (contiguous u8): even/left wins ties
                cH = pool.tile([P, R * WO], I8, tag="cH")
                cH3 = cH.rearrange("p (r w) -> p r w", r=R, w=WO)
                nc.vector.tensor_tensor(out=cH3, in0=nE, in1=nO, op=OP.is_ge)
                # horizontal norm max -> nrm odd slots (in place)
                nc.vector.tensor_tensor(out=nO, in0=nE, in1=nO, op=OP.max)

                # horizontal select: overwrite pre-filled riH with the
                # even/left candidate where it wins
                cHb = cH3.unsqueeze(1).broadcast_to([P, RI, R, WO])
                nc.vector.copy_predicated(
                    out=riH4, mask=cHb, data=xri6[:, :, :, :, 0]
                )

                # vertical mask from the horizontal maxes: bottom strictly
                # wins (top wins ties, matching argmax first-index)
                nrm5 = nrm.rearrange(
                    "p (rp rt w two) -> p rp rt w two",
                    rp=R // 2, rt=2, w=WO, two=2,
                )
                cV = pool.tile([P, (R // 2) * WO], I8, tag="cV")
                cV3 = cV.rearrange("p (rp w) -> p rp w", rp=R // 2, w=WO)
                nc.vector.tensor_tensor(
                    out=cV3,
                    in0=nrm5[:, :, 1, :, 1],
                    in1=nrm5[:, :, 0, :, 1],
                    op=OP.is_gt,
                )

                # vertical select in place: bottom row onto the top slot
                # where the bottom strictly wins; winners now sit in the
                # even rows of riH
                riH5 = riH.rearrange(
                    "p (ri rp rt w) -> p ri rp rt w",
                    ri=RI, rp=R // 2, rt=2, w=WO,
                )
                cVb = cV3.unsqueeze(1).broadcast_to([P, RI, R // 2, WO])
                nc.vector.copy_predicated(
                    out=riH5[:, :, :, 0, :], mask=cVb, data=riH5[:, :, :, 1, :]
                )

                # stream the winners straight out of riH (strided rows).
                # Issue on the SP queue: an issue on the ACT stream would
                # wait for CP-V and re-serialize square(sc+1) behind DVE.
                # Input DMAs are emitted at iteration top, so this at worst
                # delays the sc+3 input issue until CP-V(sc) — still ~1.5
                # subchunks of slack.
                o0 = sc * (R // 2)
                nc.sync.dma_start(
                    out=out[:, :, o0 : o0 + R // 2, :],
                    in_=riH5[:, :, :, 0, :],
                )
    nc.compile()
    return nc


def get_nc() -> bass.Bass:
    if not _NC_CACHE:
        _NC_CACHE.append(_build_nc())
    return _NC_CACHE[0]


def kernel(x: np.ndarray, **run_kwargs) -> np.ndarray:
    nc = get_nc()
    xs = np.asarray(x, dtype=np.float32)
    assert xs.shape == (NCORES * B, RI, C, H, W), xs.shape
    # [16,2,64,H,W] -> per core [b,c,ri,H,W] flattened to [128,ri,H,W]
    xt = np.ascontiguousarray(xs.transpose(0, 2, 1, 3, 4))
    in_maps = [
        {"x": xt[B * i : B * (i + 1)].reshape(P, RI, H, W)} for i in range(NCORES)
    ]
    res = bass_utils.run_bass_kernel_spmd(
        nc, in_maps, core_ids=list(range(NCORES)), **run_kwargs
    )
    # per-core [128,ri,HO,WO] f16 -> [b,c,ri,HO,WO] -> [b,ri,c,HO,WO]
    out = np.concatenate(
        [
            np.asarray(res.results[i]["out"])
            .reshape(B, C, RI, HO, WO)
            .transpose(0, 2, 1, 3, 4)
            for i in range(NCORES)
        ],
        axis=0,
    )
    if run_kwargs:
        kernel.last_results = res
    return np.ascontiguousarray(out.astype(np.float32))
